# revision 28
# baseline (speedup 1.0000x reference)
"""MinGRU Trainium2 kernel.

Problem: nn_MinGRU (B=8, T=4096, D=1024, fp32)
    k  = h @ W_z.T + b_z
    th = h @ W_h.T + b_h
    z = sigmoid(k);  a = 1-z = sigmoid(-k);  b = z*g(th)
    g(x) = max(x + 0.5, sigmoid(x))
    h[t] = a[t]*h[t-1] + b[t]   (fp32-state tensor_tensor_scan)

Sharding: data-parallel over batch - core i processes sample i ([T, D]).

Dataflow (v11): host pre-transposes h to [D, T] and ships it twice - bf16
(th-path) and fp8 e4m3 (z-path, weights scaled by 64, DoubleRow matmuls).
The kernel runs in 4 windows of 1024 time-steps; each matmul accumulator
is a [128, 1024] PSUM tile (2 adjacent banks), written by 512-col matmuls
and read by single 1024-wide pointwise ops. Per window, per e-pair
(2 e-tiles, matched to the weight-DMA column blocks):
  PE:     k first (8 DR matmuls per e-tile), then th (16 bf16 matmuls per
          e-tile); one DR->bf16 transition per pair. Keeping this stream
          gap-free matters twice over: the PE p-state ramps to full clock
          (~216 ns / 512-col matmul vs ~259 ns mid-state) only after ~3 us
          of continuous execution.
  Scalar: a = sig(-(k/64+bz)) and z = sig(+(k/64+bz)) both read k-PSUM;
          s = sig(th+bh) reads th-PSUM. PSUM release therefore never
          waits on Vector/GpSimd.
  Vector: g = max(th+bh+0.5, s) (the second th-PSUM reader), and the
          fp32-state scans. Scans are emitted one pair late so a pair's
          g-STTs are never queued behind its own 2.4-us scans.
  GpSimd: b = z*g only (no DMA work in steady state).
  Sync:   1024-wide output stores + startup loads + window prefetch for
          window 1; gpsimd SWDGE prefetches windows 2-3 (issued at the
          top of window w for w+2, into a long-freed bufs=3 slot).
Startup: window-0 loads are split d0-3/d4-7 and spread across the scalar
and sync HWDGE queues (SWDGE is slow to warm), weight block p arrives
just ahead of pair p.
Accuracy: fp8 on the z-path only; z = sigmoid(k) is computed directly
(mathematically identical to 1-a). Measured rel err 1.669e-2 (gate 2e-2),
identical to the v10 baseline.
"""

import contextlib
import numpy as np
import ml_dtypes
import concourse.bass as bass
import concourse.bacc as bacc
import concourse.mybir as mybir
import concourse.tile as tile
from concourse.bass_utils import run_bass_kernel_spmd

F32 = mybir.dt.float32
BF16 = mybir.dt.bfloat16
F8 = mybir.dt.float8e4
AF = mybir.ActivationFunctionType
OP = mybir.AluOpType
DR = mybir.MatmulPerfMode.DoubleRow

B, T, D = 8, 4096, 1024
NC_CORES = 8
TC = 512                 # time chunk (one fp32 PSUM bank)
TW = 2 * TC              # window = 2 chunks = 2 PSUM banks
NW = T // TW             # 4 windows
NE = D // 128            # 8 e-tiles
ND = D // 128            # 8 d-tiles
WBLK = D // 4            # weight column-block (2 e-tiles) per startup DMA
KSCALE = 1.0 / 64.0


def build_program():
    nc = bacc.Bacc("TRN2", target_bir_lowering=False, debug=False)
    # h pre-transposed on host: [D, T], in both matmul input dtypes
    hT_d = nc.dram_tensor("hT", [D, T], BF16, kind="ExternalInput").ap()
    hT8_d = nc.dram_tensor("hT8", [D, T], F8, kind="ExternalInput").ap()
    # weights pre-swizzled on host to the SBUF layout [128(dp), ND, D(e)]
    wz_d = nc.dram_tensor("wz", [128, ND, D], F8, kind="ExternalInput").ap()
    wh_d = nc.dram_tensor("wh", [128, ND, D], BF16, kind="ExternalInput").ap()
    # biases, host-precomputed: [bz, -bz, bh, bh+0.5] each [128, NE]
    bias_d = nc.dram_tensor("bias", [128, 4 * NE], F32,
                            kind="ExternalInput").ap()
    out_d = nc.dram_tensor("out", [D, T], BF16, kind="ExternalOutput").ap()

    with tile.TileContext(nc) as tc, contextlib.ExitStack() as ctx:
        const = ctx.enter_context(tc.tile_pool(name="const", bufs=1))
        # bufs=3: w+2's loads (emitted at the top of window w on gpsimd)
        # land in w-1's long-freed slot, so the DMAs fire immediately and
        # never stall the gpsimd FIFO behind them
        hTp = ctx.enter_context(tc.tile_pool(name="hT", bufs=3))
        hT8p = ctx.enter_context(tc.tile_pool(name="hT8", bufs=3))
        mmps = ctx.enter_context(tc.tile_pool(name="mmps", bufs=2, space="PSUM"))
        ew = ctx.enter_context(tc.tile_pool(name="ew", bufs=4))
        hbp = ctx.enter_context(tc.tile_pool(name="hb", bufs=2))

        # window tiles: two d-halves per dtype so startup can begin after
        # the first half lands
        hT_tiles, hT8_tiles = {}, {}

        def _h8_tile(w, half):
            t8 = hT8p.tile([128, 4, TW], F8, name=f"hT8_{w}_{half}",
                           tag=f"hT8{half}")
            src8 = bass.AP(
                tensor=hT8_d.tensor,
                offset=hT8_d.offset + half * 4 * 128 * T + w * TW,
                ap=[[T, 128], [128 * T, 4], [1, TW]],
            )
            return t8, src8

        def _hb_tile(w, half):
            tb = hTp.tile([128, 4, TW], BF16, name=f"hT{w}_{half}",
                          tag=f"hT{half}")
            srcb = bass.AP(
                tensor=hT_d.tensor,
                offset=hT_d.offset + half * 4 * 128 * T + w * TW,
                ap=[[T, 128], [128 * T, 4], [1, TW]],
            )
            return tb, srcb

        def load_window(w, eng=None):
            # 4 DMAs, fp8 halves first (k runs before th in each pair)
            eng = eng or nc.gpsimd
            t8s, tbs = [], []
            for half in range(2):
                t8, src8 = _h8_tile(w, half)
                eng.dma_start(t8, src8)
                t8s.append(t8)
            for half in range(2):
                tb, srcb = _hb_tile(w, half)
                eng.dma_start(tb, srcb)
                tbs.append(tb)
            hT_tiles[w] = tbs
            hT8_tiles[w] = t8s

        wz_sb = const.tile([128, ND, D], F8, name="wz_sb", tag="wz_sb")
        wh_sb = const.tile([128, ND, D], BF16, name="wh_sb", tag="wh_sb")

        def load_w_block(b, w_sb, src, eng):
            wsrc = bass.AP(
                tensor=src.tensor,
                offset=src.offset + b * 128 * ND * WBLK,
                ap=[[ND * WBLK, 128], [WBLK, ND], [1, WBLK]],
            )
            eng.dma_start(w_sb[:, :, b * WBLK:(b + 1) * WBLK], wsrc)

        # startup: window-0's critical loads ride the two fast HWDGE
        # queues (scalar + sync); the slow-to-warm gpsimd SWDGE queue only
        # carries window-1+ prefetch, which has tens of us of slack.
        # Consumption order in window 0: wz-b0+hT8 (k e0/e1), then
        # wh-b0+hT (th e0/e1), then block-p weights for pair p.
        # first k matmul needs wz-b0 AND h8-h0: put them on different
        # queues so they transfer in parallel
        load_w_block(0, wz_sb, wz_d, nc.sync)       # 0.25 MB
        bias_sb = const.tile([128, 4 * NE], F32)
        nc.sync.dma_start(bias_sb, bias_d)
        w0_t8, w0_tb = [], []
        for half in range(2):
            t8, src8 = _h8_tile(0, half)
            nc.scalar.dma_start(t8, src8)
            w0_t8.append(t8)
        load_w_block(0, wh_sb, wh_d, nc.scalar)     # 0.5 MB
        for half in range(2):
            tb, srcb = _hb_tile(0, half)
            (nc.sync if half == 0 else nc.scalar).dma_start(tb, srcb)
            w0_tb.append(tb)
        hT_tiles[0] = w0_tb
        hT8_tiles[0] = w0_t8
        # remaining weight blocks on sync, in consumption order
        for blk in range(1, 4):
            load_w_block(blk, wz_sb, wz_d, nc.sync)
            load_w_block(blk, wh_sb, wh_d, nc.sync)
        # window 1 behind the weights on sync (keeps the slow-to-warm
        # gpsimd SWDGE queue out of the startup-critical HBM window)
        load_window(1, eng=nc.sync)

        bz_sb = bias_sb[:, 0:NE]          # noqa: F841  (kept for reference)
        negbz = bias_sb[:, NE:2 * NE]
        bh_sb = bias_sb[:, 2 * NE:3 * NE]
        bh05 = bias_sb[:, 3 * NE:4 * NE]

        prev_hb = [None] * NE
        pending_scans = []

        for w in range(NW):
            hTb = hT_tiles.pop(w)
            hT8b = hT8_tiles.pop(w)
            if w + 2 < NW:
                load_window(w + 2)

            for p in range(NE // 2):
                pair = (2 * p, 2 * p + 1)
                if w == NW - 1 and p == NE // 2 - 1:
                    # last pair: process e7 (Vector-b) first so e6's
                    # GpSimd-b chain overlaps Vector's scan in the tail
                    pair = (2 * p + 1, 2 * p)
                kps_t, thps_t = {}, {}

                # K phase: fp8 DoubleRow, both chunks back-to-back per
                # weight tile so the stationary operand can be reused
                for e in pair:
                    es = slice(e * 128, (e + 1) * 128)
                    k_ps = mmps.tile([128, TW], F32, name=f"k{w}_{e}",
                                     tag="k", bufs=2)
                    for dp in range(ND // 2):
                        lhs = wz_sb[:, 2 * dp:2 * dp + 2, es]
                        t8 = hT8b[dp // 2]
                        dl = dp % 2
                        for c in range(2):
                            nc.tensor.matmul(
                                k_ps[:, c * TC:(c + 1) * TC], lhs,
                                t8[:, 2 * dl:2 * dl + 2, c * TC:(c + 1) * TC],
                                start=(dp == 0), stop=(dp == ND // 2 - 1),
                                perf_mode=DR)
                    kps_t[e] = k_ps

                # TH phase: bf16, chunk-paired per weight tile
                for e in pair:
                    es = slice(e * 128, (e + 1) * 128)
                    th_ps = mmps.tile([128, TW], F32, name=f"th{w}_{e}",
                                      tag="th", bufs=2)
                    for d in range(ND):
                        lhs = wh_sb[:, d, es]
                        tb = hTb[d // 4]
                        dl = d % 4
                        for c in range(2):
                            nc.tensor.matmul(
                                th_ps[:, c * TC:(c + 1) * TC], lhs,
                                tb[:, dl, c * TC:(c + 1) * TC],
                                start=(d == 0), stop=(d == ND - 1))
                    thps_t[e] = th_ps

                # deferred scans from the previous pair go first on the
                # Vector queue: they are ready (their b's are long done) and
                # must not sit in front of this pair's g-STTs, which release
                # the th-PSUM ring
                for fn in pending_scans:
                    fn()
                pending_scans = []

                # pointwise, 1024 wide. k-PSUM is released by the two
                # Scalar ACTs (a+z); th-PSUM by s (Scalar) + g (Vector).
                at, zt, st, gt = {}, {}, {}, {}
                for e in pair:
                    at[e] = ew.tile([128, TW], F32, name=f"a{w}_{e}", tag="a",
                                    bufs=4)
                    nc.scalar.activation(at[e], kps_t[e], AF.Sigmoid,
                                         bias=negbz[:, e:e + 1],
                                         scale=-KSCALE)
                    # z = sigmoid(+k/64 + bz) (== 1-a), straight from PSUM
                    zt[e] = ew.tile([128, TW], F32, name=f"z{w}_{e}", tag="z",
                                    bufs=4)
                    nc.scalar.activation(zt[e], kps_t[e], AF.Sigmoid,
                                         bias=bz_sb[:, e:e + 1],
                                         scale=KSCALE)
                for e in pair:
                    st[e] = ew.tile([128, TW], F32, name=f"s{w}_{e}", tag="s",
                                    bufs=4)
                    nc.scalar.activation(st[e], thps_t[e], AF.Sigmoid,
                                         bias=bh_sb[:, e:e + 1])
                for e in pair:
                    # g = max(th + bh + 0.5, s)
                    gt[e] = ew.tile([128, TW], F32, name=f"g{w}_{e}", tag="g",
                                    bufs=3)
                    nc.vector.scalar_tensor_tensor(
                        gt[e], thps_t[e], bh05[:, e:e + 1], st[e],
                        op0=OP.add, op1=OP.max)
                for e in pair:
                    # b = z * g; e1 on Vector: its b then follows its g
                    # in-order, cutting the cross-engine hop out of the
                    # pair-critical chain s(e1)->g(e1)->b(e1)->scans
                    b_t = ew.tile([128, TW], F32, name=f"b{w}_{e}", tag="b",
                                  bufs=3)
                    beng = nc.gpsimd if e % 2 == 0 else nc.vector
                    beng.tensor_tensor(b_t, zt[e], gt[e], OP.mult)

                    def mk_scan(w=w, e=e, a_t=at[e], b_t=b_t):
                        hb = hbp.tile([128, TW], BF16, name=f"hb{w}_{e}",
                                      tag=f"hb{e}")
                        init = 0.0 if w == 0 else prev_hb[e][:, TW - 1:TW]
                        nc.vector.tensor_tensor_scan(hb, a_t, b_t, init,
                                                     OP.mult, OP.add)
                        prev_hb[e] = hb
                        dst = bass.AP(
                            tensor=out_d.tensor,
                            offset=out_d.offset + e * 128 * T + w * TW,
                            ap=[[T, 128], [1, TW]],
                        )
                        nc.sync.dma_start(dst, hb)
                    pending_scans.append(mk_scan)
                kps_t.clear()
                thps_t.clear()
        # flush the last pair's deferred scans
        for fn in pending_scans:
            fn()

    nc.compile()
    return nc


_nc_cache = None


def _get_program():
    global _nc_cache
    if _nc_cache is None:
        _nc_cache = build_program()
    return _nc_cache


def _make_in_maps(h_prev_layer, W_z, b_z, W_h, b_h):
    bf = ml_dtypes.bfloat16
    f8 = ml_dtypes.float8_e4m3

    # lhsT layout [d, e], swizzled to [4 blocks][128 dp][ND dt][blk e] -
    # per-partition contiguous per block
    def swizzle(W, dtype=bf, scale=1.0):
        wT = np.ascontiguousarray(W.T.astype(np.float32) * scale)  # [d, e]
        w = wT.reshape(ND, 128, 4, WBLK).transpose(2, 1, 0, 3)
        return np.ascontiguousarray(w.astype(dtype))

    wzq = swizzle(W_z, f8, 64.0)
    whq = swizzle(W_h)
    bz8 = b_z.reshape(NE, 128).T.astype(np.float32)
    bh8 = b_h.reshape(NE, 128).T.astype(np.float32)
    bias = np.ascontiguousarray(
        np.concatenate([bz8, -bz8, bh8, bh8 + 0.5], axis=1))
    maps = []
    for i in range(B):
        hTf = np.ascontiguousarray(h_prev_layer[i].T.astype(np.float32))
        m = {
            "hT": hTf.astype(bf),
            "hT8": hTf.astype(f8),
            "wz": wzq, "wh": whq, "bias": bias,
        }
        maps.append(m)
    return maps


def run(inputs, trace=False, **kw):
    nc = _get_program()
    in_maps = _make_in_maps(**inputs)
    res = run_bass_kernel_spmd(nc, in_maps, core_ids=list(range(NC_CORES)),
                               trace=trace, **kw)
    # device output is [D, T] bf16; un-transpose + upcast on host
    out = np.stack([res.results[i]["out"].T.astype(np.float32)
                    for i in range(NC_CORES)], axis=0)
    return out, res


def kernel(h_prev_layer, W_z, b_z, W_h, b_h):
    out, _ = run(dict(h_prev_layer=h_prev_layer, W_z=W_z, b_z=b_z,
                      W_h=W_h, b_h=b_h))
    return out


# revision 30
# speedup vs baseline: 1.0014x; 1.0014x over previous
"""MinGRU Trainium2 kernel.

Problem: nn_MinGRU (B=8, T=4096, D=1024, fp32)
    k  = h @ W_z.T + b_z
    th = h @ W_h.T + b_h
    z = sigmoid(k);  a = 1-z = sigmoid(-k);  b = z*g(th)
    g(x) = max(x + 0.5, sigmoid(x))
    h[t] = a[t]*h[t-1] + b[t]   (fp32-state tensor_tensor_scan)

Sharding: data-parallel over batch - core i processes sample i ([T, D]).

Dataflow (v11): host pre-transposes h to [D, T] and ships it twice - bf16
(th-path) and fp8 e4m3 (z-path, weights scaled by 64, DoubleRow matmuls).
The kernel runs in 4 windows of 1024 time-steps; each matmul accumulator
is a [128, 1024] PSUM tile (2 adjacent banks), written by 512-col matmuls
and read by single 1024-wide pointwise ops. Per window, per e-pair
(2 e-tiles, matched to the weight-DMA column blocks):
  PE:     k first (8 DR matmuls per e-tile), then th (16 bf16 matmuls per
          e-tile); one DR->bf16 transition per pair. Keeping this stream
          gap-free matters twice over: the PE p-state ramps to full clock
          (~216 ns / 512-col matmul vs ~259 ns mid-state) only after ~3 us
          of continuous execution.
  Scalar: a = sig(-(k/64+bz)) and z = sig(+(k/64+bz)) both read k-PSUM;
          s = sig(th+bh) reads th-PSUM. PSUM release therefore never
          waits on Vector/GpSimd.
  Vector: g = max(th+bh+0.5, s) (the second th-PSUM reader), and the
          fp32-state scans. Scans are emitted one pair late so a pair's
          g-STTs are never queued behind its own 2.4-us scans.
  GpSimd: b = z*g only (no DMA work in steady state).
  Sync:   1024-wide output stores + startup loads + window prefetch for
          window 1; gpsimd SWDGE prefetches windows 2-3 (issued at the
          top of window w for w+2, into a long-freed bufs=3 slot).
Startup: window-0 loads are split d0-3/d4-7 and spread across the scalar
and sync HWDGE queues (SWDGE is slow to warm), weight block p arrives
just ahead of pair p.
Accuracy: fp8 on the z-path only; z = sigmoid(k) is computed directly
(mathematically identical to 1-a). Measured rel err 1.669e-2 (gate 2e-2),
identical to the v10 baseline.
"""

import contextlib
import numpy as np
import ml_dtypes
import concourse.bass as bass
import concourse.bacc as bacc
import concourse.mybir as mybir
import concourse.tile as tile
from concourse.bass_utils import run_bass_kernel_spmd

F32 = mybir.dt.float32
BF16 = mybir.dt.bfloat16
F8 = mybir.dt.float8e4
AF = mybir.ActivationFunctionType
OP = mybir.AluOpType
DR = mybir.MatmulPerfMode.DoubleRow

B, T, D = 8, 4096, 1024
NC_CORES = 8
TC = 512                 # time chunk (one fp32 PSUM bank)
TW = 2 * TC              # window = 2 chunks = 2 PSUM banks
NW = T // TW             # 4 windows
NE = D // 128            # 8 e-tiles
ND = D // 128            # 8 d-tiles
WBLK = D // 4            # weight column-block (2 e-tiles) per startup DMA
KSCALE = 1.0 / 64.0


def build_program():
    nc = bacc.Bacc("TRN2", target_bir_lowering=False, debug=False)
    # h pre-transposed on host: [D, T], in both matmul input dtypes
    hT_d = nc.dram_tensor("hT", [D, T], BF16, kind="ExternalInput").ap()
    hT8_d = nc.dram_tensor("hT8", [D, T], F8, kind="ExternalInput").ap()
    # weights pre-swizzled on host to the SBUF layout [128(dp), ND, D(e)]
    wz_d = nc.dram_tensor("wz", [128, ND, D], F8, kind="ExternalInput").ap()
    wh_d = nc.dram_tensor("wh", [128, ND, D], BF16, kind="ExternalInput").ap()
    # biases, host-precomputed: [bz, -bz, bh, bh+0.5] each [128, NE]
    bias_d = nc.dram_tensor("bias", [128, 4 * NE], F32,
                            kind="ExternalInput").ap()
    out_d = nc.dram_tensor("out", [D, T], BF16, kind="ExternalOutput").ap()

    with tile.TileContext(nc) as tc, contextlib.ExitStack() as ctx:
        const = ctx.enter_context(tc.tile_pool(name="const", bufs=1))
        # bufs=3: w+2's loads (emitted at the top of window w on gpsimd)
        # land in w-1's long-freed slot, so the DMAs fire immediately and
        # never stall the gpsimd FIFO behind them
        hTp = ctx.enter_context(tc.tile_pool(name="hT", bufs=3))
        hT8p = ctx.enter_context(tc.tile_pool(name="hT8", bufs=3))
        mmps = ctx.enter_context(tc.tile_pool(name="mmps", bufs=2, space="PSUM"))
        ew = ctx.enter_context(tc.tile_pool(name="ew", bufs=4))
        hbp = ctx.enter_context(tc.tile_pool(name="hb", bufs=2))

        # window tiles: two d-halves per dtype so startup can begin after
        # the first half lands
        hT_tiles, hT8_tiles = {}, {}

        def _h8_tile(w, half):
            t8 = hT8p.tile([128, 4, TW], F8, name=f"hT8_{w}_{half}",
                           tag=f"hT8{half}")
            src8 = bass.AP(
                tensor=hT8_d.tensor,
                offset=hT8_d.offset + half * 4 * 128 * T + w * TW,
                ap=[[T, 128], [128 * T, 4], [1, TW]],
            )
            return t8, src8

        def _hb_tile(w, half):
            tb = hTp.tile([128, 4, TW], BF16, name=f"hT{w}_{half}",
                          tag=f"hT{half}")
            srcb = bass.AP(
                tensor=hT_d.tensor,
                offset=hT_d.offset + half * 4 * 128 * T + w * TW,
                ap=[[T, 128], [128 * T, 4], [1, TW]],
            )
            return tb, srcb

        def load_window(w, eng=None):
            # 4 DMAs, fp8 halves first (k runs before th in each pair)
            eng = eng or nc.gpsimd
            t8s, tbs = [], []
            for half in range(2):
                t8, src8 = _h8_tile(w, half)
                eng.dma_start(t8, src8)
                t8s.append(t8)
            for half in range(2):
                tb, srcb = _hb_tile(w, half)
                eng.dma_start(tb, srcb)
                tbs.append(tb)
            hT_tiles[w] = tbs
            hT8_tiles[w] = t8s

        wz_sb = const.tile([128, ND, D], F8, name="wz_sb", tag="wz_sb")
        wh_sb = const.tile([128, ND, D], BF16, name="wh_sb", tag="wh_sb")

        def load_w_block(b, w_sb, src, eng):
            wsrc = bass.AP(
                tensor=src.tensor,
                offset=src.offset + b * 128 * ND * WBLK,
                ap=[[ND * WBLK, 128], [WBLK, ND], [1, WBLK]],
            )
            eng.dma_start(w_sb[:, :, b * WBLK:(b + 1) * WBLK], wsrc)

        # startup: window-0's critical loads ride the two fast HWDGE
        # queues (scalar + sync); the slow-to-warm gpsimd SWDGE queue only
        # carries window-1+ prefetch, which has tens of us of slack.
        # Consumption order in window 0: wz-b0+hT8 (k e0/e1), then
        # wh-b0+hT (th e0/e1), then block-p weights for pair p.
        # first k matmul needs wz-b0 AND h8-h0: put them on different
        # queues so they transfer in parallel
        load_w_block(0, wz_sb, wz_d, nc.sync)       # 0.25 MB
        bias_sb = const.tile([128, 4 * NE], F32)
        nc.sync.dma_start(bias_sb, bias_d)
        w0_t8, w0_tb = [], []
        for half in range(2):
            t8, src8 = _h8_tile(0, half)
            nc.scalar.dma_start(t8, src8)
            w0_t8.append(t8)
        load_w_block(0, wh_sb, wh_d, nc.scalar)     # 0.5 MB
        for half in range(2):
            tb, srcb = _hb_tile(0, half)
            (nc.sync if half == 0 else nc.scalar).dma_start(tb, srcb)
            w0_tb.append(tb)
        hT_tiles[0] = w0_tb
        hT8_tiles[0] = w0_t8
        # remaining weight blocks on sync, in consumption order
        for blk in range(1, 4):
            load_w_block(blk, wz_sb, wz_d, nc.sync)
            load_w_block(blk, wh_sb, wh_d, nc.sync)
        # window 1 behind the weights on sync (keeps the slow-to-warm
        # gpsimd SWDGE queue out of the startup-critical HBM window)
        load_window(1, eng=nc.sync)

        bz_sb = bias_sb[:, 0:NE]          # noqa: F841  (kept for reference)
        negbz = bias_sb[:, NE:2 * NE]
        bh_sb = bias_sb[:, 2 * NE:3 * NE]
        bh05 = bias_sb[:, 3 * NE:4 * NE]

        prev_hb = [None] * NE
        pending_scans = []

        for w in range(NW):
            hTb = hT_tiles.pop(w)
            hT8b = hT8_tiles.pop(w)
            if w + 2 < NW:
                load_window(w + 2)

            for p in range(NE // 2):
                pair = (2 * p, 2 * p + 1)
                if w == NW - 1 and p == NE // 2 - 1:
                    # last pair: process e7 (Vector-b) first so e6's
                    # GpSimd-b chain overlaps Vector's scan in the tail
                    pair = (2 * p + 1, 2 * p)
                kps_t, thps_t = {}, {}

                # K phase: fp8 DoubleRow, both chunks back-to-back per
                # weight tile so the stationary operand can be reused
                for e in pair:
                    es = slice(e * 128, (e + 1) * 128)
                    k_ps = mmps.tile([128, TW], F32, name=f"k{w}_{e}",
                                     tag="k", bufs=2)
                    for dp in range(ND // 2):
                        lhs = wz_sb[:, 2 * dp:2 * dp + 2, es]
                        t8 = hT8b[dp // 2]
                        dl = dp % 2
                        for c in range(2):
                            nc.tensor.matmul(
                                k_ps[:, c * TC:(c + 1) * TC], lhs,
                                t8[:, 2 * dl:2 * dl + 2, c * TC:(c + 1) * TC],
                                start=(dp == 0), stop=(dp == ND // 2 - 1),
                                perf_mode=DR)
                    kps_t[e] = k_ps

                # TH phase: bf16, chunk-paired per weight tile
                for e in pair:
                    es = slice(e * 128, (e + 1) * 128)
                    th_ps = mmps.tile([128, TW], F32, name=f"th{w}_{e}",
                                      tag="th", bufs=2)
                    for d in range(ND):
                        lhs = wh_sb[:, d, es]
                        tb = hTb[d // 4]
                        dl = d % 4
                        for c in range(2):
                            nc.tensor.matmul(
                                th_ps[:, c * TC:(c + 1) * TC], lhs,
                                tb[:, dl, c * TC:(c + 1) * TC],
                                start=(d == 0), stop=(d == ND - 1))
                    thps_t[e] = th_ps

                # deferred scans from the previous pair go first on the
                # Vector queue: they are ready (their b's are long done) and
                # must not sit in front of this pair's g-STTs, which release
                # the th-PSUM ring
                for fn in pending_scans:
                    fn()
                pending_scans = []

                # pointwise, 1024 wide. k-PSUM is released by the two
                # Scalar ACTs (a+z); th-PSUM by s (Scalar) + g (Vector).
                at, zt, st, gt = {}, {}, {}, {}
                for e in pair:
                    at[e] = ew.tile([128, TW], F32, name=f"a{w}_{e}", tag="a",
                                    bufs=4)
                    nc.scalar.activation(at[e], kps_t[e], AF.Sigmoid,
                                         bias=negbz[:, e:e + 1],
                                         scale=-KSCALE)
                    # z = sigmoid(+k/64 + bz) (== 1-a), straight from PSUM
                    zt[e] = ew.tile([128, TW], F32, name=f"z{w}_{e}", tag="z",
                                    bufs=4)
                    nc.scalar.activation(zt[e], kps_t[e], AF.Sigmoid,
                                         bias=bz_sb[:, e:e + 1],
                                         scale=KSCALE)
                for e in pair:
                    st[e] = ew.tile([128, TW], F32, name=f"s{w}_{e}", tag="s",
                                    bufs=4)
                    nc.scalar.activation(st[e], thps_t[e], AF.Sigmoid,
                                         bias=bh_sb[:, e:e + 1])
                for e in pair:
                    # g = max(th + bh + 0.5, s)
                    gt[e] = ew.tile([128, TW], F32, name=f"g{w}_{e}", tag="g",
                                    bufs=3)
                    nc.vector.scalar_tensor_tensor(
                        gt[e], thps_t[e], bh05[:, e:e + 1], st[e],
                        op0=OP.add, op1=OP.max)
                for e in pair:
                    # b = z * g; e1 on Vector: its b then follows its g
                    # in-order, cutting the cross-engine hop out of the
                    # pair-critical chain s(e1)->g(e1)->b(e1)->scans
                    b_t = ew.tile([128, TW], F32, name=f"b{w}_{e}", tag="b",
                                  bufs=3)
                    beng = nc.gpsimd if e % 2 == 0 else nc.vector
                    beng.tensor_tensor(b_t, zt[e], gt[e], OP.mult)

                    def mk_scan(w=w, e=e, a_t=at[e], b_t=b_t):
                        hb = hbp.tile([128, TW], BF16, name=f"hb{w}_{e}",
                                      tag=f"hb{e}")
                        init = 0.0 if w == 0 else prev_hb[e][:, TW - 1:TW]
                        nc.vector.tensor_tensor_scan(hb, a_t, b_t, init,
                                                     OP.mult, OP.add)
                        prev_hb[e] = hb
                        dst = bass.AP(
                            tensor=out_d.tensor,
                            offset=out_d.offset + e * 128 * T + w * TW,
                            ap=[[T, 128], [1, TW]],
                        )
                        nc.sync.dma_start(dst, hb)
                    pending_scans.append(mk_scan)
                kps_t.clear()
                thps_t.clear()
        # flush the last pair's deferred scans
        for fn in pending_scans:
            fn()

    nc.compile()
    return nc


_nc_cache = None


def _get_program():
    global _nc_cache
    if _nc_cache is None:
        _nc_cache = build_program()
    return _nc_cache


def _make_in_maps(h_prev_layer, W_z, b_z, W_h, b_h):
    bf = ml_dtypes.bfloat16
    f8 = ml_dtypes.float8_e4m3

    # lhsT layout [d, e], swizzled to [4 blocks][128 dp][ND dt][blk e] -
    # per-partition contiguous per block
    def swizzle(W, dtype=bf, scale=1.0):
        wT = np.ascontiguousarray(W.T.astype(np.float32) * scale)  # [d, e]
        w = wT.reshape(ND, 128, 4, WBLK).transpose(2, 1, 0, 3)
        return np.ascontiguousarray(w.astype(dtype))

    wzq = swizzle(W_z, f8, 64.0)
    whq = swizzle(W_h)
    bz8 = b_z.reshape(NE, 128).T.astype(np.float32)
    bh8 = b_h.reshape(NE, 128).T.astype(np.float32)
    bias = np.ascontiguousarray(
        np.concatenate([bz8, -bz8, bh8, bh8 + 0.5], axis=1))
    maps = []
    for i in range(B):
        hTf = np.ascontiguousarray(h_prev_layer[i].T.astype(np.float32))
        m = {
            "hT": hTf.astype(bf),
            "hT8": hTf.astype(f8),
            "wz": wzq, "wh": whq, "bias": bias,
        }
        maps.append(m)
    return maps


def run(inputs, trace=False, **kw):
    nc = _get_program()
    in_maps = _make_in_maps(**inputs)
    res = run_bass_kernel_spmd(nc, in_maps, core_ids=list(range(NC_CORES)),
                               trace=trace, **kw)
    # device output is [D, T] bf16; un-transpose + upcast on host
    out = np.stack([res.results[i]["out"].T.astype(np.float32)
                    for i in range(NC_CORES)], axis=0)
    return out, res


def kernel(h_prev_layer, W_z, b_z, W_h, b_h):
    out, _ = run(dict(h_prev_layer=h_prev_layer, W_z=W_z, b_z=b_z,
                      W_h=W_h, b_h=b_h))
    return out
